# revision 105
# baseline (speedup 1.0000x reference)
"""Causal self-attention (B=8, T=1024, C=768, H=12, Dh=64) on 8 TRN2 NeuronCores.

Sharding: batch data-parallel. Core b computes the full attention block for
batch element b (weights replicated). No collectives.

Per-core dataflow (fp32 data; matmuls run as float32r at full rate by keeping
every moving dim >= 256):
  1. x [T,C] -> xT [C,T] via PE transposes (f32r, 1.5 cyc/row).
  2. Q^T,K^T [C,T] = W^T @ xT per head-pair; V [T, C] = x @ W_v stored per
     head with an all-ones 65th column (V_aug) so the P@V matmul also
     accumulates softmax denominators.
  3. Per head, per k-block kb: S^T spans [k=128, q] over the causal range;
     P^T = exp(S^T/8) on ACT per 512-span; causal zeroing via affine_select
     on GPSIMD; O'^T [65, q] += V_aug^T @ P^T into per-half PSUM banks.
     Denominator row 64 -> DVE reciprocal -> GPSIMD partition_broadcast ->
     DVE multiply normalizes into OT [C, T].
  4. y [T,C] = OT-as-lhsT @ W_out + b_out, DMA to DRAM.

Scheduling: the PE issues in order, so the stream is software-pipelined to
avoid micro-idles (which drop the HAM clock gate to 1.2 GHz): P@V for block
kb issues after S^T for block kb+1, and the QKV/output projections are
decomposed into single-matmul "filler" closures interleaved between S and PV
so the PE queue always holds dependency-free work while ACT/GPSIMD chew on
exp/select. V projection chains stream as fillers through heads 0-1; pair
j+1's Q/K projection streams through pair j's heads.
"""

import numpy as np
from collections import deque

B, T, C = 8, 1024, 768
H, D = 12, 64
TB = T // 128  # 8 t/k blocks
CB = C // 128  # 6 channel blocks
NCORES = 8

_CACHE = {}


def _ensure_path():
    import sys

    for p in ("/opt/trn_rl_repo",):
        if p not in sys.path:
            sys.path.insert(0, p)


def _emit(nc, tc, tile, mybir, make_identity):
    f32 = mybir.dt.float32
    f32r = mybir.dt.float32r
    bf16 = mybir.dt.bfloat16
    Exp = mybir.ActivationFunctionType.Exp
    Ln = mybir.ActivationFunctionType.Ln
    isge = mybir.AluOpType.is_ge

    x_d = nc.dram_tensor("x", [T, C], f32r, kind="ExternalInput")
    wqkv_d = nc.dram_tensor("W_qkv", [C, 3 * C], f32, kind="ExternalInput")
    bqkv_d = nc.dram_tensor("b_qkv", [3 * C], f32, kind="ExternalInput")
    wout_d = nc.dram_tensor("W_out", [C, C], f32, kind="ExternalInput")
    bout_d = nc.dram_tensor("b_out", [C], f32, kind="ExternalInput")
    y_d = nc.dram_tensor("y_out", [T, C], f32, kind="ExternalOutput")

    with (
        tc.tile_pool(name="const", bufs=1) as const_pool,
        tc.tile_pool(name="wres", bufs=1) as wres,
        tc.tile_pool(name="wqkp", bufs=3) as wqk_pool,
        tc.tile_pool(name="xin", bufs=6) as xin_pool,
        tc.tile_pool(name="big", bufs=1) as big,
        tc.tile_pool(name="qktp", bufs=2) as qkt_pool,
        tc.tile_pool(name="ptp", bufs=4) as pt_pool,
        tc.tile_pool(name="yp", bufs=3) as y_pool,
        tc.tile_pool(name="smallp", bufs=2) as small_pool,
        tc.tile_pool(name="mmp", bufs=2, space="PSUM") as mm_psum,
        tc.tile_pool(name="stp", bufs=3, space="PSUM") as st_psum,
        tc.tile_pool(name="ot0p", bufs=1, space="PSUM") as ot0_psum,
        tc.tile_pool(name="ot1p", bufs=2, space="PSUM") as ot1_psum,
    ):
        # ---------- constants ----------
        ident_f = const_pool.tile([128, 128], f32, name="ident_f")
        make_identity(nc, ident_f[:])
        ident = const_pool.tile([128, 128], f32r, name="ident")
        nc.vector.tensor_copy(ident[:], ident_f[:])
        identr = ident[:]

        # Load the Exp activation table while the PE is still in startup
        # (first real exp otherwise pays ~1.3us mid-attention).
        scratch = const_pool.tile([1, 2], f32, name="scratch")
        nc.gpsimd.memset(scratch[:], 0.0)
        nc.scalar.activation(scratch[0:1, 0:1], scratch[0:1, 1:2], Exp)

        zero_fill = nc.gpsimd.to_reg(0.0)

        xT = big.tile([128, CB, T], bf16, name="xT")
        V = big.tile([128, TB, H, D + 1], bf16, name="V")
        OT = [big.tile([128, T], bf16, name=f"OT{cb}", tag=f"OT{cb}") for cb in range(CB)]
        ypart = big.tile([128, 16, 384], f32, name="ypart")
        nc.gpsimd.memset(V[:, :, :, D : D + 1], 1.0)

        # ---------- DMA priority order ----------
        # SP queue: x0-x5, wqk0, x6-x7 (pool-gated), wqk1; wqk j+2 issued at
        # pair-j start. ACT queue: tiny biases first, then wv, wout last.
        x_in = [xin_pool.tile([128, C], f32r, name="x_in", tag="x_in") for _ in range(TB)]

        def issue_x(tb):
            nc.sync.dma_start(x_in[tb][:], x_d[tb * 128 : (tb + 1) * 128, :])

        # weights land as raw fp32 and are cast to bf16 on DVE (bf16
        # stationaries enable FWL so LDWEIGHTS hides; bf16 matmuls run full
        # rate at any moving width). Pairs are loaded two-at-a-time in
        # 256-wide slices: DMA here is descriptor-throughput bound, and 1KB
        # lines halve the descriptor count vs 512B.
        wqk_raw = {}
        wqk_tiles = {}

        def issue_wqk(j):
            raw = wqk_pool.tile([128, CB, 2, 128], f32, name="wqkr", tag="wqkr", bufs=3)
            for qk in range(2):
                for cb in range(CB):
                    nc.sync.dma_start(
                        raw[:, cb, qk, :],
                        wqkv_d[
                            cb * 128 : (cb + 1) * 128,
                            qk * C + j * 128 : qk * C + (j + 1) * 128,
                        ],
                    )
            wqk_raw[j] = raw

        def cast_wqk(j):
            wqk_tiles[j] = wqk_pool.tile(
                [128, CB, 2, 128], bf16, name="wqk", tag="wqkb", bufs=3
            )
            nc.vector.tensor_copy(wqk_tiles[j][:], wqk_raw.pop(j)[:])

        for tb in range(4):
            issue_x(tb)
        issue_wqk(0)
        # wv chunks 2-3 + wout issued from SP after the priority traffic so
        # their ring descriptors queue behind x/wqk0/wqk1
        wv_r = wres.tile([128, CB, C], f32, name="wv_r")
        wv = wres.tile([128, CB, C], bf16, name="wv")
        wout_r = wres.tile([128, CB, C], f32, name="wout_r")
        wout = wres.tile([128, CB, C], bf16, name="wout")
        for tb in range(4, TB):
            issue_x(tb)
        for cb in range(CB):
            nc.sync.dma_start(
                wv_r[:, cb, 0:384],
                wqkv_d[cb * 128 : (cb + 1) * 128, 2 * C : 2 * C + 384],
            )
        issue_wqk(1)
        for cb in range(CB):
            nc.sync.dma_start(
                wv_r[:, cb, 384:768],
                wqkv_d[cb * 128 : (cb + 1) * 128, 2 * C + 384 : 3 * C],
            )
        for cb in range(CB):
            nc.sync.dma_start(wout_r[:, cb, :], wout_d[cb * 128 : (cb + 1) * 128, :])

        def cast_wv(half):
            nc.vector.tensor_copy(
                wv[:, :, half * 384 : (half + 1) * 384],
                wv_r[:, :, half * 384 : (half + 1) * 384],
            )

        def cast_wout(half):
            nc.vector.tensor_copy(
                wout[:, :, half * 384 : (half + 1) * 384],
                wout_r[:, :, half * 384 : (half + 1) * 384],
            )

        # b_qkv Q/K part as [128, 12]: column m holds channels m*128..m*128+127.
        # Loaded contiguously as [12, 128] (12 descriptors instead of ~1500
        # 4-byte ones) and PE-transposed during startup.
        bqk18 = const_pool.tile([12, 128], f32, name="bqk18")
        nc.scalar.dma_start(bqk18[:], bqkv_d[0 : 12 * 128].rearrange("(m p) -> m p", p=128))
        bqk = const_pool.tile([128, 12], f32, name="bqk")

        bv_bc = const_pool.tile([128, C], f32, name="bv_bc")
        nc.scalar.dma_start(bv_bc[0:1, :], bqkv_d[2 * C : 3 * C][None, :])
        nc.gpsimd.partition_broadcast(bv_bc[:], bv_bc[0:1, :])

        bo_bc = const_pool.tile([128, C], f32, name="bo_bc")
        nc.scalar.dma_start(bo_bc[0:1, :], bout_d[:][None, :])
        nc.gpsimd.partition_broadcast(bo_bc[:], bo_bc[0:1, :])



        # ---------- transposes: x -> xT ----------
        def transpose_tile(tb):
            xi = x_in[tb][:]
            ps_a = mm_psum.tile([128, 512], f32, name="ps_a", tag="mm")
            for i in range(4):
                nc.tensor.transpose(
                    ps_a[:, i * 128 : (i + 1) * 128].bitcast(f32r),
                    xi[:, i * 128 : (i + 1) * 128],
                    identr,
                )
            nc.vector.tensor_copy(
                xT[:, 0:4, tb * 128 : (tb + 1) * 128],
                ps_a[:].rearrange("p (c t) -> p c t", c=4),
            )
            ps_b = mm_psum.tile([128, 512], f32, name="ps_b", tag="mm")
            for i in range(2):
                cb = 4 + i
                nc.tensor.transpose(
                    ps_b[:, i * 128 : (i + 1) * 128].bitcast(f32r),
                    xi[:, cb * 128 : (cb + 1) * 128],
                    identr,
                )
            nc.vector.tensor_copy(
                xT[:, 4:6, tb * 128 : (tb + 1) * 128],
                ps_b[:, 0:256].rearrange("p (c t) -> p c t", c=2),
            )

        # ---------- Q/K projection groups ----------
        qkt_tiles = {}

        def emit_qk_group(j, qk, tch, state):
            # one accumulation step; allocate psum lazily at first call
            def mk(cb):
                def g():
                    if "ps" not in state:
                        state["ps"] = mm_psum.tile([128, 512], f32, name="ps_qk", tag="mm")
                    nc.tensor.matmul(
                        state["ps"][:],
                        wqk_tiles[j][:, cb, qk, :],
                        xT[:, cb, tch * 512 : (tch + 1) * 512],
                        start=(cb == 0),
                        stop=(cb == CB - 1),
                    )
                return g

            def fin():
                m_idx = qk * 6 + j
                nc.vector.tensor_scalar_add(
                    qkt_tiles[j][:, qk, tch * 512 : (tch + 1) * 512],
                    state["ps"][:],
                    bqk[:, m_idx : m_idx + 1],
                )

            return [mk(cb) for cb in range(CB)] + [fin]

        def qkproj_closures(j):
            # cb-major with the two t-halves paired: consecutive matmuls
            # share the stationary wqk[cb, qk], amortizing LDWEIGHTS; the
            # two open accumulation groups use exactly the 2-deep mm ring.
            # The bf16 weight cast is split per qk half so the first matmuls
            # wait only half a cast and the second half's cast hides behind
            # them.
            def mk_cast(qk):
                def c():
                    if j not in wqk_tiles:
                        wqk_tiles[j] = wqk_pool.tile(
                            [128, CB, 2, 128], bf16, name="wqk", tag="wqkb", bufs=3
                        )
                    nc.vector.tensor_copy(
                        wqk_tiles[j][:, :, qk, :], wqk_raw[j][:, :, qk, :]
                    )
                return c

            out = []
            for qk in range(2):
                out.append(mk_cast(qk))
                states = ({}, {})

                def mk(cb, tch, qk=qk, states=states):
                    def g():
                        st = states[tch]
                        if "ps" not in st:
                            st["ps"] = mm_psum.tile(
                                [128, 512], f32, name="ps_qk", tag="mm"
                            )
                        nc.tensor.matmul(
                            st["ps"][:],
                            wqk_tiles[j][:, cb, qk, :],
                            xT[:, cb, tch * 512 : (tch + 1) * 512],
                            start=(cb == 0),
                            stop=(cb == CB - 1),
                        )
                    return g

                def fin(tch, qk=qk, states=states):
                    def f():
                        m_idx = qk * 6 + j
                        nc.vector.tensor_scalar_add(
                            qkt_tiles[j][:, qk, tch * 512 : (tch + 1) * 512],
                            states[tch]["ps"][:],
                            bqk[:, m_idx : m_idx + 1],
                        )
                    return f

                for cb in range(CB):
                    out.append(mk(cb, 0))
                    out.append(mk(cb, 1))
                out.append(fin(0))
                out.append(fin(1))
            return out

        # ---------- V projection chains (384-wide: 6 heads per chunk) ----------
        def vproj_closures(tb, ch):
            state = {}

            def mk(cb):
                def g():
                    if "ps" not in state:
                        state["ps"] = mm_psum.tile([128, 512], f32, name="ps_v", tag="mm")
                    nc.tensor.matmul(
                        state["ps"][:, 0:384],
                        xT[:, cb, tb * 128 : (tb + 1) * 128],
                        wv[:, cb, ch * 384 : (ch + 1) * 384],
                        start=(cb == 0),
                        stop=(cb == CB - 1),
                    )
                return g

            def fin():
                nc.vector.tensor_add(
                    V[:, tb, ch * 6 : (ch + 1) * 6, 0:D],
                    state["ps"][:, 0:384].rearrange("p (h d) -> p h d", h=6),
                    bv_bc[:, ch * 384 : (ch + 1) * 384].rearrange("p (h d) -> p h d", h=6),
                )

            return [mk(cb) for cb in range(CB)] + [fin]

        # ---------- startup emission ----------
        for tb in range(4):
            transpose_tile(tb)
        ps_bq = mm_psum.tile([128, 512], f32, name="ps_bq", tag="mm")
        nc.tensor.transpose(ps_bq[:, 0:12], bqk18[:], ident_f[0:12, 0:12])
        nc.vector.tensor_copy(bqk[:], ps_bq[:, 0:12])
        # pair 0: stream the bf16 cast per (qk, cb) sub-tile so the first
        # QK0 matmul starts as soon as its own DMA slice lands, instead of
        # waiting for the whole descriptor-bound 0.77MB load
        wqk_tiles[0] = wqk_pool.tile([128, CB, 2, 128], bf16, name="wqk", tag="wqkb", bufs=3)
        raw0 = wqk_raw.pop(0)
        for qk in range(2):
            for cb in range(CB):
                nc.vector.tensor_copy(
                    wqk_tiles[0][:, cb, qk, :], raw0[:, cb, qk, :]
                )
        # Q/K proj j=0, t-half 0 (needs x tiles 0-3 + wqk0)
        qkt_tiles[0] = qkt_pool.tile([128, 2, T], bf16, name="qkt", tag="qkt")
        for cl in emit_qk_group(0, 0, 0, {}) + emit_qk_group(0, 1, 0, {}):
            cl()
        for tb in range(4, TB):
            transpose_tile(tb)
        cast_wv(0)
        for cl in emit_qk_group(0, 0, 1, {}) + emit_qk_group(0, 1, 1, {}):
            cl()
        # ---------- output projection closures ----------
        # Stage 1 accumulates the pair blocks already finished into SBUF
        # partials (bias pre-added): tb0-3 use cb [0,1,2,3] (legal from
        # position 8), tb4-7 use cb [0,1,2,3,5] (position 10). Stage 2 adds
        # the remaining pair contributions at the very end.
        def outproj_stage1(tb, ch, cbs):
            state = {}

            def mk(k):
                def g():
                    if "ps" not in state:
                        state["ps"] = mm_psum.tile([128, 512], f32, name="ps_y", tag="mm")
                    cb = cbs[k]
                    nc.tensor.matmul(
                        state["ps"][:, 0:384],
                        OT[cb][:, tb * 128 : (tb + 1) * 128],
                        wout[:, cb, ch * 384 : (ch + 1) * 384],
                        start=(k == 0),
                        stop=(k == len(cbs) - 1),
                    )
                return g

            def fin():
                nc.vector.tensor_add(
                    ypart[:, tb * 2 + ch, :],
                    state["ps"][:, 0:384],
                    bo_bc[:, ch * 384 : (ch + 1) * 384],
                )

            return [mk(k) for k in range(len(cbs))] + [fin]

        def outproj_stage2(tb, cbs):
            tiles = {}

            def mkch(ch):
                state = {}

                def mk(k):
                    def g():
                        if "ps" not in state:
                            state["ps"] = mm_psum.tile(
                                [128, 512], f32, name="ps_y2", tag="mm"
                            )
                        cb = cbs[k]
                        nc.tensor.matmul(
                            state["ps"][:, 0:384],
                            OT[cb][:, tb * 128 : (tb + 1) * 128],
                            wout[:, cb, ch * 384 : (ch + 1) * 384],
                            start=(k == 0),
                            stop=(k == len(cbs) - 1),
                        )
                    return g

                def fin():
                    if "yt" not in tiles:
                        tiles["yt"] = y_pool.tile([128, C], f32, name="yt", tag="yt")
                    nc.vector.tensor_add(
                        tiles["yt"][:, ch * 384 : (ch + 1) * 384],
                        state["ps"][:, 0:384],
                        ypart[:, tb * 2 + ch, :],
                    )

                return [mk(k) for k in range(len(cbs))] + [fin]

            def dma():
                nc.sync.dma_start(y_d[tb * 128 : (tb + 1) * 128, :], tiles["yt"][:])

            return mkch(0) + mkch(1) + [dma]

        def outproj_accum(tb, ch, cb):
            # single-cb accumulation into the SBUF partial (in-place add)
            state = {}

            def mm():
                state["ps"] = mm_psum.tile([128, 512], f32, name="ps_ya", tag="mm")
                nc.tensor.matmul(
                    state["ps"][:, 0:384],
                    OT[cb][:, tb * 128 : (tb + 1) * 128],
                    wout[:, cb, ch * 384 : (ch + 1) * 384],
                    start=True,
                    stop=True,
                )

            def fin():
                nc.vector.tensor_add(
                    ypart[:, tb * 2 + ch, :],
                    state["ps"][:, 0:384],
                    ypart[:, tb * 2 + ch, :],
                )

            return [mm, fin]

        # first four V chains inline: head 0's first PVs need them, and they
        # soak up the DMA-gated startup window where the PE would idle
        for tb in range(4):
            for cl in vproj_closures(tb, 0):
                cl()

        # ---------- filler queue ----------
        filler = deque()
        for tb in range(4, TB):
            filler.extend(vproj_closures(tb, 0))

        def pop_fillers(n):
            c = 0
            while filler and c < n:
                filler.popleft()()
                c += 1

        # ---------- attention heads, software-pipelined ----------
        def head_spans(kb):
            # list of (b2, c0, c1): exact causal per-PSUM-half column spans
            # (bf16 matmuls run full rate at any moving width)
            v0 = kb * 128
            spans = []
            for b2 in range(kb // 4, 2):
                blo = b2 * 512
                spans.append((b2, max(v0, blo), blo + 512))
            return spans

        def emit_S(stt, kb):
            h, i, qkt_t = stt["h"], stt["i"], stt["qkt"]
            if kb == 0:
                stt["ot"] = [
                    ot0_psum.tile([D + 1, 512], f32, name="ot0", tag="ot0"),
                    ot1_psum.tile([D + 1, 512], f32, name="ot1", tag="ot1"),
                ]
            v0 = kb * 128
            pt = pt_pool.tile([128, T], bf16, name="pt", tag="pt")
            stt["pt"][kb] = pt
            kq = qkt_t[i * 64 : (i + 1) * 64, 1, kb * 128 : (kb + 1) * 128]
            spans = head_spans(kb)
            stt["spans"][kb] = spans
            # high half (no mask) first so its exp unlocks PV early
            for b2, c0, c1 in reversed(spans):
                w = c1 - c0
                stc = st_psum.tile([128, 512], f32, name="st", tag="st")
                nc.tensor.matmul(
                    stc[:, 0:w],
                    kq,
                    qkt_t[i * 64 : (i + 1) * 64, 0, c0:c1],
                    start=True,
                    stop=True,
                )
                nc.scalar.activation(pt[:, c0:c1], stc[:, 0:w], Exp, scale=0.125)
            # causal zeroing on the diagonal block: [v0, v0+128) valid iff
            # q >= v0 + p
            nc.gpsimd.affine_select(
                out=pt[:, v0 : v0 + 128],
                in_=pt[:, v0 : v0 + 128],
                compare_op=isge,
                fill=zero_fill,
                base=0,
                channel_multiplier=-1,
                pattern=[[1, 128]],
            )

        def emit_PV(stt, kb):
            h = stt["h"]
            pt = stt["pt"].pop(kb)
            for b2, c0, c1 in reversed(stt["spans"].pop(kb)):
                nc.tensor.matmul(
                    stt["ot"][b2][:, c0 - b2 * 512 : 512],
                    V[:, kb, h, :],
                    pt[:, c0:c1],
                    start=(kb == 0),
                    stop=(kb == 4 * b2 + 3),
                )

        def emit_norm(stt, qc):
            # 1/s = exp(-ln s) on ACT (same pinned table set)
            j, i = stt["j"], stt["i"]
            ot = stt["ot"][qc]
            lns = small_pool.tile([1, 512], f32, name="lns", tag="lns")
            nc.scalar.activation(lns[:], ot[D : D + 1, :], Ln)
            recip = small_pool.tile([1, 512], f32, name="recip", tag="recip")
            nc.scalar.activation(recip[:], lns[:], Exp, scale=-1.0)
            rbc = small_pool.tile([64, 512], f32, name="rbc", tag="rbc")
            nc.gpsimd.partition_broadcast(rbc[:], recip[:])
            nc.vector.tensor_mul(
                OT[j][i * 64 : (i + 1) * 64, qc * 512 : (qc + 1) * 512],
                ot[0:D, :],
                rbc[:],
            )

        # Pair processing order: pair 4 last so the output projection (cb
        # order [0,1,2,3,5,4]) can start inside the final head.
        pair_seq = [0, 1, 2, 3, 5, 4]

        def push_qkproj(j):
            qkt_tiles[j] = qkt_pool.tile([128, 2, T], bf16, name="qkt", tag="qkt")
            filler.extend(qkproj_closures(j))

        QUOTAS = [4, 4, 4, 3, 3, 3, 3, 3, 5, 5, 4, 4]
        prev = None
        for pos in range(H):
            j = pair_seq[pos // 2]
            i = pos % 2
            h = 2 * j + i
            if pos == 0:
                push_qkproj(1)
                issue_wqk(2)
            elif pos == 1:
                issue_wqk(3)
                filler.append(lambda: cast_wv(1))
                for tb in range(4):
                    filler.extend(vproj_closures(tb, 1))
            elif pos == 2:
                push_qkproj(2)
            elif pos == 3:
                push_qkproj(3)
                issue_wqk(5)
            elif pos == 5:
                for tb in range(4, TB):
                    filler.extend(vproj_closures(tb, 1))
                push_qkproj(5)
            elif pos == 4:
                filler.append(lambda: cast_wout(0))
                filler.append(lambda: cast_wout(1))
            elif pos == 6:
                issue_wqk(4)
            elif pos == 8:
                push_qkproj(4)
            quota = QUOTAS[pos]
            stt = {"h": h, "j": j, "i": i, "qkt": qkt_tiles[j], "pt": {}, "spans": {}}
            for kb in range(TB):
                emit_S(stt, kb)
                if pos == 5 and kb == 5:
                    for tb in range(4):
                        filler.extend(outproj_stage1(tb, 0, [0, 1, 2]))
                        filler.extend(outproj_stage1(tb, 1, [0, 1, 2]))
                elif pos == 7 and kb == 5:
                    for tb in range(4):
                        filler.extend(outproj_accum(tb, 0, 3))
                        filler.extend(outproj_accum(tb, 1, 3))
                elif pos == 9 and kb == 5:
                    for tb in range(4, TB):
                        filler.extend(outproj_stage1(tb, 0, [0, 1, 2, 3]))
                        filler.extend(outproj_stage1(tb, 1, [0, 1, 2, 3]))
                elif pos == 10 and kb == 1:
                    for tb in range(4, TB):
                        filler.extend(outproj_accum(tb, 0, 5))
                        filler.extend(outproj_accum(tb, 1, 5))
                elif pos == H - 1 and kb == 5:
                    for tb in range(4):
                        filler.extend(outproj_stage2(tb, [5, 4]))
                pop_fillers(8 if (pos == H - 1 and kb >= 5) else quota)
                if kb == 0:
                    if prev is not None:
                        emit_PV(prev, 7)
                        emit_norm(prev, 1)
                else:
                    emit_PV(stt, kb - 1)
                    if kb - 1 == 3:
                        emit_norm(stt, 0)
            prev = stt
        emit_PV(prev, 7)
        emit_norm(prev, 1)
        pop_fillers(1 << 30)
        for tb in range(4, TB):
            for cl in outproj_stage2(tb, [4]):
                cl()


def build():
    if "nc" in _CACHE:
        return _CACHE["nc"]
    _ensure_path()
    import concourse.bacc as bacc
    import concourse.mybir as mybir
    import concourse.tile as tile
    from concourse.masks import make_identity

    nc = bacc.Bacc(
        "TRN2",
        target_bir_lowering=False,
        debug=False,
        enable_asserts=False,
        num_devices=NCORES,
    )
    with tile.TileContext(nc) as tc:
        _emit(nc, tc, tile, mybir, make_identity)

    # Both Exp and Ln live in the 'natural_log_exp_and_others' ACT table set,
    # but the table-load pass maps Exp to the first set containing it
    # ('exp_and_others'), so Exp/Ln ping-pong table loads every head
    # (~1.3us each).  Restrict Exp membership to the natural_log set for the
    # duration of compile; dict order (= act_func_set_id) is preserved.
    orig_tables = bacc.get_activation_tables

    def _pinned_tables(arch):
        tables = orig_tables(arch)
        exp_t = mybir.ActivationFunctionType.Exp
        if any(exp_t in fns for name, fns in tables.items() if "natural_log" in name):
            for name, fns in tables.items():
                if "natural_log" not in name:
                    fns.discard(exp_t)
        return tables

    bacc.get_activation_tables = _pinned_tables
    try:
        nc.compile()
    finally:
        bacc.get_activation_tables = orig_tables
    _CACHE["nc"] = nc
    return nc


def _in_maps(x, W_qkv, b_qkv, W_out, b_out):
    x = np.ascontiguousarray(np.asarray(x, dtype=np.float32))
    W_qkv = np.ascontiguousarray(np.asarray(W_qkv, dtype=np.float32))
    b_qkv = np.ascontiguousarray(np.asarray(b_qkv, dtype=np.float32))
    W_out = np.ascontiguousarray(np.asarray(W_out, dtype=np.float32))
    b_out = np.ascontiguousarray(np.asarray(b_out, dtype=np.float32))
    return [
        {
            "x": x[b],
            "W_qkv": W_qkv,
            "b_qkv": b_qkv,
            "W_out": W_out,
            "b_out": b_out,
        }
        for b in range(B)
    ]


def _install_ntff_hook():
    """The image's antenv package lacks axon_hooks; synthesize it so
    run_bass_kernel_spmd(trace=True) can NTFF-profile via libaxon_pjrt.so."""
    import sys
    import types

    if "antenv.axon_hooks" in sys.modules:
        return
    mod = types.ModuleType("antenv.axon_hooks")
    state = {"hook": None}
    mod.set_axon_ntff_profile_hook = lambda h: state.__setitem__("hook", h)
    mod.get_axon_ntff_profile_hook = lambda: state["hook"]
    sys.modules["antenv.axon_hooks"] = mod
    import antenv

    antenv.axon_hooks = mod
    try:
        if "/root/.axon_site" not in sys.path:
            sys.path.append("/root/.axon_site")
        from trn_agent_boot.trn_boot import _ntff_profile_via_ctypes

        mod.set_axon_ntff_profile_hook(
            _ntff_profile_via_ctypes("/opt/axon/libaxon_pjrt.so")
        )
    except Exception as exc:  # degrade to no tracing
        print(f"ntff hook unavailable: {exc}", file=sys.stderr)


def run(x, W_qkv, b_qkv, W_out, b_out, trace=False):
    _ensure_path()
    if trace:
        _install_ntff_hook()
    from concourse.bass_utils import run_bass_kernel_spmd

    nc = build()
    res = run_bass_kernel_spmd(
        nc,
        _in_maps(x, W_qkv, b_qkv, W_out, b_out),
        core_ids=list(range(NCORES)),
        trace=trace,
    )
    y = np.stack([res.results[b]["y_out"] for b in range(B)], axis=0)
    return y.astype(np.float32, copy=False), res


def kernel(x, W_qkv, b_qkv, W_out, b_out):
    y, _ = run(x, W_qkv, b_qkv, W_out, b_out, trace=False)
    return y
